# revision 19
# baseline (speedup 1.0000x reference)
"""DLinear forward folded to one mat-vec, 8-bit quantized, on 8 TRN2 cores.

The reference network is linear in x:
    out[b] = sum_f x[b,f] * v[f] + const
with v folding the moving-average, the per-channel linears and the decoder
(computed on host in float64 — weights only, tiny).

The 662MB x dominates: the kernel is HBM-bandwidth bound, so x is quantized
to 8-bit on host (4x less device traffic than f32; the dequant scales fold
into v). Features are sharded across the 8 cores (each core owns a
contiguous 10112-feature slice of the transposed x and all 2048 batch
columns); each core computes a partial dot product and the host sums the 8
partials (plus the folded constant) in float64.

Every byte moves exactly once as a 1-byte element (1MB per-quad DMAs
alternating the qSP HWDGE ring and the SWDGE ring — both triggered from
engines that do no compute, so triggers never queue behind work). Each quad
is owned entirely by one of three compute lanes, with per-lane tile pools so
a slow lane never blocks another lane's buffers:
 - e3 quads (fp8 e3m4 bytes, x*2 with the scale folded into v): the PE
   streams fp8 at full rate against the bf16 v-chunk [128,1] stationary,
   accumulating into psum [1,512]x4 across chunks. These sit at the end of
   the stream: the PE drains a chunk in 0.86us, keeping the tail short.
 - act quads (int8, clip 4 sigma): one fused ACT op converts the whole quad
   int8->bf16 (ints <= 127 are bf16-exact), the PE consumes.
 - dve quads (int8): scalar_tensor_tensor accumulates
   z_acc[p,b] += x[p,b]*v[p]; a ones-matmul partition-reduces z_acc into
   spare psum banks, ACT copies it out, and a final DVE add merges the two
   psum halves into y.
int8 carries ~0.0094 relative error and e3m4 ~0.018; with ~1/3 of features
on e3m4 the measured end-to-end l2 error is ~1.3e-2 against the 2e-2 gate.
"""

import sys

import numpy as np

for _p in ("/opt/trn_rl_repo",):
    if _p not in sys.path:
        sys.path.insert(0, _p)

_B, _L, _C = 2048, 512, 158
_K = 25
_PAD = (_K - 1) // 2
_NCORES = 8
_F = _L * _C                    # 80896 features
_FSH = _F // _NCORES            # 10112 features per core
_NCH = _FSH // 128              # 79 chunks of 128 features
_NCHP = 80                      # padded to 80 chunks (last one all-zero v)
_NOCT = _NCHP // 8              # 10 oct-tiles per core (host layout unit)
_NQALL = _NCHP // 4             # 20 quads (quad 19 holds the pad chunk 79)
_CLIP = 4.0
_QSCALE = 127.0 / _CLIP         # int8 scale
_E3_SCALE = 2.0                 # fp8 e3m4 scale (max |2x| ~ 11.4 < 15.5)

# quad -> lane: ACT/DVE interleaved up front, fp8/PE quads at the tail
_QLANE = (["act", "dve"] * 6 + ["act"])[:13] + ["pe8"] * 7
assert len(_QLANE) == _NQALL


def _fold_weights(w_seasonal, b_seasonal, w_trend, b_trend, w_dec, b_dec):
    w_s = np.asarray(w_seasonal, np.float64)
    w_t = np.asarray(w_trend, np.float64)
    b_s = np.asarray(b_seasonal, np.float64)
    b_t = np.asarray(b_trend, np.float64)
    w_d = np.asarray(w_dec, np.float64)
    b_d = float(np.asarray(b_dec, np.float64))
    C, L = w_s.shape
    # M[l, lp] = #{d in [-p, p] : clamp(l+d, 0, L-1) == lp}: the linear map of
    # the edge-padded moving average, so that sum_l trend[.,l]*g[l] ==
    # sum_lp x[.,lp] * (g @ M)[lp] / K exactly.
    M = np.zeros((L, L))
    for l in range(L):
        for d in range(-_PAD, _PAD + 1):
            M[l, min(max(l + d, 0), L - 1)] += 1.0
    Wcomb = w_s + ((w_t - w_s) @ M) / _K        # [C, L]
    W = Wcomb * w_d[:, None]                    # [C, L]
    v = np.ascontiguousarray(W.T).reshape(-1)   # index l*C+c, float64
    const = float(np.sum(w_d * (b_s + b_t)) + b_d)
    return v, const


def _build():
    from contextlib import ExitStack

    import concourse.bacc as bacc
    import concourse.mybir as mybir
    import concourse.tile as tile

    f32 = mybir.dt.float32
    bf16 = mybir.dt.bfloat16
    i8 = mybir.dt.int8
    f8e3 = mybir.dt.float8e3

    nc = bacc.Bacc(None, target_bir_lowering=False)
    xq = nc.dram_tensor("xq", [_NOCT, 128, 8 * _B], i8, kind="ExternalInput")
    vpe = nc.dram_tensor("vpe", [128, _NCHP], bf16, kind="ExternalInput")
    vdve = nc.dram_tensor("vdve", [128, _NCHP], f32, kind="ExternalInput")
    y = nc.dram_tensor("y", [1, _B], f32, kind="ExternalOutput")

    pe_chunks = [ci for ci in range(_NCH) if _QLANE[ci // 4] != "dve"]
    first_pe, last_pe = min(pe_chunks), max(pe_chunks)
    dve_chunks = [ci for ci in range(_NCH) if _QLANE[ci // 4] == "dve"]
    first_dve, last_dve = min(dve_chunks), max(dve_chunks)

    with tile.TileContext(nc) as tc, ExitStack() as ctx:
        epool = ctx.enter_context(tc.tile_pool(name="ep", bufs=4))
        apool = ctx.enter_context(tc.tile_pool(name="ap", bufs=4))
        dpool = ctx.enter_context(tc.tile_pool(name="dp", bufs=4))
        cpool = ctx.enter_context(tc.tile_pool(name="cp", bufs=3))
        ppool = ctx.enter_context(tc.tile_pool(name="pp", bufs=1, space="PSUM"))
        spool = ctx.enter_context(tc.tile_pool(name="sp", bufs=1))

        vpe_t = spool.tile([128, _NCHP], bf16)
        vdve_t = spool.tile([128, _NCHP], f32)
        ones = spool.tile([128, 1], f32)
        z_acc = spool.tile([128, _B], f32)
        zy_sb = spool.tile([1, _B], f32)
        y_sb = spool.tile([1, _B], f32)
        nc.sync.dma_start(out=vpe_t, in_=vpe[:, :])
        nc.sync.dma_start(out=vdve_t, in_=vdve[:, :])
        nc.vector.memset(ones, 1.0)

        ppsum = ppool.tile([1, 4 * 512], f32)
        zpsum = ppool.tile([1, 4 * 512], f32)

        def pe_mms(xs, ci):
            for j in range(4):
                nc.tensor.matmul(
                    ppsum[0:1, j * 512:(j + 1) * 512],
                    vpe_t[:, ci:ci + 1],
                    xs[:, j * 512:(j + 1) * 512],
                    start=(ci == first_pe), stop=(ci == last_pe and j == 3),
                )

        def do_dve(xs, ci):
            if ci == first_dve:
                nc.vector.tensor_scalar(
                    out=z_acc, in0=xs,
                    scalar1=vdve_t[:, ci:ci + 1], scalar2=None,
                    op0=mybir.AluOpType.mult,
                )
            else:
                nc.vector.scalar_tensor_tensor(
                    out=z_acc, in0=xs,
                    scalar=vdve_t[:, ci:ci + 1], in1=z_acc,
                    op0=mybir.AluOpType.mult, op1=mybir.AluOpType.add,
                )
            if ci == last_dve:
                # partition-reduce z_acc into the spare psum banks, then
                # stage to SBUF via ACT (all overlapping the fp8 tail)
                for j in range(4):
                    nc.tensor.matmul(
                        zpsum[0:1, j * 512:(j + 1) * 512], ones,
                        z_acc[:, j * 512:(j + 1) * 512],
                        start=True, stop=True,
                    )
                nc.scalar.copy(out=zy_sb, in_=zpsum)

        dmas = [nc.sync, nc.gpsimd]
        for q in range(_NQALL):
            o, h0 = q // 2, 4 * (q % 2)
            nch = 3 if q == _NQALL - 1 else 4
            kind = _QLANE[q]
            pool = {"pe8": epool, "act": apool, "dve": dpool}[kind]
            rt = pool.tile([128, 4, _B], i8, name=f"t{kind}")
            dmas[q % 2].dma_start(
                out=rt[:, :nch, :],
                in_=xq[o:o + 1, :, h0 * _B:(h0 + nch) * _B],
            )
            if kind == "pe8":
                for h in range(nch):
                    pe_mms(rt[:, h, :].bitcast(f8e3), 4 * q + h)
            elif kind == "act":
                cv = cpool.tile([128, 4, _B], bf16)
                nc.scalar.copy(out=cv[:, :nch, :], in_=rt[:, :nch, :])
                for h in range(nch):
                    pe_mms(cv[:, h, :], 4 * q + h)
            else:
                for h in range(nch):
                    do_dve(rt[:, h, :], 4 * q + h)

        # y = ppsum + zy_sb (PSUM + SBUF operands — legal on the DVE)
        nc.vector.tensor_tensor(
            out=y_sb, in0=ppsum, in1=zy_sb, op=mybir.AluOpType.add,
        )
        nc.sync.dma_start(out=y[:, :], in_=y_sb)
    nc.compile()
    return nc


def kernel(**inputs):
    import ml_dtypes

    x = np.asarray(inputs["x"], dtype=np.float32)
    assert x.shape == (_B, _L, _C), x.shape
    v, const = _fold_weights(
        inputs["w_seasonal"], inputs["b_seasonal"],
        inputs["w_trend"], inputs["b_trend"],
        inputs["w_dec"], inputs["b_dec"],
    )

    xT = np.ascontiguousarray(x.reshape(_B, _F).T)          # [F, B] f32
    e3_chunks = [ci for ci in range(_NCH) if _QLANE[ci // 4] == "pe8"]

    nc = _build()

    from concourse.bass_utils import run_bass_kernel_spmd

    in_maps = []
    for c in range(_NCORES):
        sh = xT[c * _FSH:(c + 1) * _FSH]                    # [10112, B] f32
        shp = np.zeros((_NCHP * 128, _B), np.int8)
        shp[:_FSH] = np.clip(
            np.rint(sh * _QSCALE), -127, 127).astype(np.int8)
        vs = np.zeros(_NCHP * 128, np.float64)
        vs[:_FSH] = v[c * _FSH:(c + 1) * _FSH] / _QSCALE
        for ci in e3_chunks:
            r0 = ci * 128
            shp[r0:r0 + 128] = (
                sh[r0:r0 + 128] * _E3_SCALE
            ).astype(ml_dtypes.float8_e3m4).view(np.int8)
            vs[r0:r0 + 128] = v[c * _FSH + r0:c * _FSH + r0 + 128] / _E3_SCALE
        # [oct, chunk-in-oct, partition, batch] -> [oct, partition, ...]
        xqc = np.ascontiguousarray(
            shp.reshape(_NOCT, 8, 128, _B).transpose(0, 2, 1, 3)
        ).reshape(_NOCT, 128, 8 * _B)
        vmat = np.ascontiguousarray(vs.reshape(_NCHP, 128).T)   # [128, NCHP]
        in_maps.append({
            "xq": xqc,
            "vpe": vmat.astype(ml_dtypes.bfloat16),
            "vdve": vmat.astype(np.float32),
        })
    r = run_bass_kernel_spmd(nc, in_maps, core_ids=list(range(_NCORES)))
    kernel._last = r
    acc = np.zeros(_B, np.float64)
    for i in range(_NCORES):
        acc += r.results[i]["y"].reshape(-1).astype(np.float64)
    return (acc + const).astype(np.float32)


# revision 21
# speedup vs baseline: 1.1402x; 1.1402x over previous
"""DLinear forward folded to one mat-vec, 8-bit quantized, on 8 TRN2 cores.

The reference network is linear in x:
    out[b] = sum_f x[b,f] * v[f] + const
with v folding the moving-average, the per-channel linears and the decoder
(computed on host in float64 — weights only, tiny).

The 662MB x dominates: the kernel is HBM-bandwidth bound, so x is quantized
to 8-bit on host (4x less device traffic than f32; the dequant scales fold
into v). Features are sharded across the 8 cores (each core owns a
contiguous 10112-feature slice of the transposed x and all 2048 batch
columns); each core computes a partial dot product and the host sums the 8
partials (plus the folded constant) in float64.

Every byte moves exactly once as a 1-byte element (1MB per-quad DMAs
alternating the qSP HWDGE ring and the SWDGE ring — both triggered from
engines that do no compute, so triggers never queue behind work). Each quad
is owned entirely by one of three compute lanes, with per-lane tile pools so
a slow lane never blocks another lane's buffers:
 - e3 quads (fp8 e3m4 bytes, x*2 with the scale folded into v): the PE
   streams fp8 at full rate against the bf16 v-chunk [128,1] stationary,
   accumulating into psum [1,512]x4 across chunks. These sit at the end of
   the stream: the PE drains a chunk in 0.86us, keeping the tail short.
 - act quads (int8, clip 4 sigma): one fused ACT op converts the whole quad
   int8->bf16 (ints <= 127 are bf16-exact), the PE consumes.
 - dve quads (int8): scalar_tensor_tensor accumulates
   z_acc[p,b] += x[p,b]*v[p]; a ones-matmul partition-reduces z_acc into
   spare psum banks, ACT copies it out, and a final DVE add merges the two
   psum halves into y.
int8 carries ~0.0094 relative error and e3m4 ~0.018; with ~1/3 of features
on e3m4 the measured end-to-end l2 error is ~1.3e-2 against the 2e-2 gate.
"""

import sys

import numpy as np

for _p in ("/opt/trn_rl_repo",):
    if _p not in sys.path:
        sys.path.insert(0, _p)

_B, _L, _C = 2048, 512, 158
_K = 25
_PAD = (_K - 1) // 2
_NCORES = 8
_F = _L * _C                    # 80896 features
_FSH = _F // _NCORES            # 10112 features per core
_NCH = _FSH // 128              # 79 chunks of 128 features
_NCHP = 80                      # padded to 80 chunks (last one all-zero v)
_NOCT = _NCHP // 8              # 10 oct-tiles per core (host layout unit)
_NQALL = _NCHP // 4             # 20 quads (quad 19 holds the pad chunk 79)
_CLIP = 4.0
_QSCALE = 127.0 / _CLIP         # int8 scale
_E3_SCALE = 2.0                 # fp8 e3m4 scale (max |2x| ~ 11.4 < 15.5)

# quad -> lane, interleaved so every engine gets steady work in stream
# order (the PE queue is FIFO: fp8 and ACT-fed matmuls must arrive in the
# same order their data does); the last two quads are fp8 (fast drain)
_QLANE = ["act", "dve", "pe8"] * 6 + ["pe8", "pe8"]
assert len(_QLANE) == _NQALL


def _fold_weights(w_seasonal, b_seasonal, w_trend, b_trend, w_dec, b_dec):
    w_s = np.asarray(w_seasonal, np.float64)
    w_t = np.asarray(w_trend, np.float64)
    b_s = np.asarray(b_seasonal, np.float64)
    b_t = np.asarray(b_trend, np.float64)
    w_d = np.asarray(w_dec, np.float64)
    b_d = float(np.asarray(b_dec, np.float64))
    C, L = w_s.shape
    # M[l, lp] = #{d in [-p, p] : clamp(l+d, 0, L-1) == lp}: the linear map of
    # the edge-padded moving average, so that sum_l trend[.,l]*g[l] ==
    # sum_lp x[.,lp] * (g @ M)[lp] / K exactly.
    M = np.zeros((L, L))
    for l in range(L):
        for d in range(-_PAD, _PAD + 1):
            M[l, min(max(l + d, 0), L - 1)] += 1.0
    Wcomb = w_s + ((w_t - w_s) @ M) / _K        # [C, L]
    W = Wcomb * w_d[:, None]                    # [C, L]
    v = np.ascontiguousarray(W.T).reshape(-1)   # index l*C+c, float64
    const = float(np.sum(w_d * (b_s + b_t)) + b_d)
    return v, const


def _build():
    from contextlib import ExitStack

    import concourse.bacc as bacc
    import concourse.mybir as mybir
    import concourse.tile as tile

    f32 = mybir.dt.float32
    bf16 = mybir.dt.bfloat16
    i8 = mybir.dt.int8
    f8e3 = mybir.dt.float8e3

    nc = bacc.Bacc(None, target_bir_lowering=False)
    xq = nc.dram_tensor("xq", [_NOCT, 128, 8 * _B], i8, kind="ExternalInput")
    vpe = nc.dram_tensor("vpe", [128, _NCHP], bf16, kind="ExternalInput")
    vdve = nc.dram_tensor("vdve", [128, _NCHP], f32, kind="ExternalInput")
    y = nc.dram_tensor("y", [1, _B], f32, kind="ExternalOutput")

    pe_chunks = [ci for ci in range(_NCH) if _QLANE[ci // 4] != "dve"]
    first_pe, last_pe = min(pe_chunks), max(pe_chunks)
    dve_chunks = [ci for ci in range(_NCH) if _QLANE[ci // 4] == "dve"]
    first_dve, last_dve = min(dve_chunks), max(dve_chunks)

    with tile.TileContext(nc) as tc, ExitStack() as ctx:
        epool = ctx.enter_context(tc.tile_pool(name="ep", bufs=4))
        apool = ctx.enter_context(tc.tile_pool(name="ap", bufs=4))
        dpool = ctx.enter_context(tc.tile_pool(name="dp", bufs=4))
        cpool = ctx.enter_context(tc.tile_pool(name="cp", bufs=3))
        ppool = ctx.enter_context(tc.tile_pool(name="pp", bufs=1, space="PSUM"))
        spool = ctx.enter_context(tc.tile_pool(name="sp", bufs=1))

        vpe_t = spool.tile([128, _NCHP], bf16)
        vdve_t = spool.tile([128, _NCHP], f32)
        ones = spool.tile([128, 1], f32)
        z_acc = spool.tile([128, _B], f32)
        zy_sb = spool.tile([1, _B], f32)
        y_sb = spool.tile([1, _B], f32)
        nc.sync.dma_start(out=vpe_t, in_=vpe[:, :])
        nc.sync.dma_start(out=vdve_t, in_=vdve[:, :])
        nc.vector.memset(ones, 1.0)

        ppsum = ppool.tile([1, 4 * 512], f32)
        zpsum = ppool.tile([1, 4 * 512], f32)

        def pe_mms(xs, ci):
            for j in range(4):
                nc.tensor.matmul(
                    ppsum[0:1, j * 512:(j + 1) * 512],
                    vpe_t[:, ci:ci + 1],
                    xs[:, j * 512:(j + 1) * 512],
                    start=(ci == first_pe), stop=(ci == last_pe and j == 3),
                )

        def do_dve(xs, ci):
            if ci == first_dve:
                nc.vector.tensor_scalar(
                    out=z_acc, in0=xs,
                    scalar1=vdve_t[:, ci:ci + 1], scalar2=None,
                    op0=mybir.AluOpType.mult,
                )
            else:
                nc.vector.scalar_tensor_tensor(
                    out=z_acc, in0=xs,
                    scalar=vdve_t[:, ci:ci + 1], in1=z_acc,
                    op0=mybir.AluOpType.mult, op1=mybir.AluOpType.add,
                )
            if ci == last_dve:
                # partition-reduce z_acc into the spare psum banks, then
                # stage to SBUF via ACT (all overlapping the fp8 tail)
                for j in range(4):
                    nc.tensor.matmul(
                        zpsum[0:1, j * 512:(j + 1) * 512], ones,
                        z_acc[:, j * 512:(j + 1) * 512],
                        start=True, stop=True,
                    )
                nc.scalar.copy(out=zy_sb, in_=zpsum)

        # one DMA ring per lane: act quads ride qSP (sync triggers instantly),
        # dve quads ride SWDGE (Q7 only emits descriptors), fp8 quads ride
        # qAct (triggers slot between ACT converts); ~150 GB/s per ring
        dmas = {"act": nc.sync, "dve": nc.gpsimd, "pe8": nc.scalar}
        for q in range(_NQALL):
            o, h0 = q // 2, 4 * (q % 2)
            nch = 3 if q == _NQALL - 1 else 4
            kind = _QLANE[q]
            pool = {"pe8": epool, "act": apool, "dve": dpool}[kind]
            rt = pool.tile([128, 4, _B], i8, name=f"t{kind}")
            dmas[kind].dma_start(
                out=rt[:, :nch, :],
                in_=xq[o:o + 1, :, h0 * _B:(h0 + nch) * _B],
            )
            if kind == "pe8":
                for h in range(nch):
                    pe_mms(rt[:, h, :].bitcast(f8e3), 4 * q + h)
            elif kind == "act":
                cv = cpool.tile([128, 4, _B], bf16)
                nc.scalar.copy(out=cv[:, :nch, :], in_=rt[:, :nch, :])
                for h in range(nch):
                    pe_mms(cv[:, h, :], 4 * q + h)
            else:
                for h in range(nch):
                    do_dve(rt[:, h, :], 4 * q + h)

        # y = ppsum + zy_sb (PSUM + SBUF operands — legal on the DVE)
        nc.vector.tensor_tensor(
            out=y_sb, in0=ppsum, in1=zy_sb, op=mybir.AluOpType.add,
        )
        nc.sync.dma_start(out=y[:, :], in_=y_sb)
    nc.compile()
    return nc


def kernel(**inputs):
    import ml_dtypes

    x = np.asarray(inputs["x"], dtype=np.float32)
    assert x.shape == (_B, _L, _C), x.shape
    v, const = _fold_weights(
        inputs["w_seasonal"], inputs["b_seasonal"],
        inputs["w_trend"], inputs["b_trend"],
        inputs["w_dec"], inputs["b_dec"],
    )

    xT = np.ascontiguousarray(x.reshape(_B, _F).T)          # [F, B] f32
    e3_chunks = [ci for ci in range(_NCH) if _QLANE[ci // 4] == "pe8"]

    nc = _build()

    from concourse.bass_utils import run_bass_kernel_spmd

    in_maps = []
    for c in range(_NCORES):
        sh = xT[c * _FSH:(c + 1) * _FSH]                    # [10112, B] f32
        shp = np.zeros((_NCHP * 128, _B), np.int8)
        shp[:_FSH] = np.clip(
            np.rint(sh * _QSCALE), -127, 127).astype(np.int8)
        vs = np.zeros(_NCHP * 128, np.float64)
        vs[:_FSH] = v[c * _FSH:(c + 1) * _FSH] / _QSCALE
        for ci in e3_chunks:
            r0 = ci * 128
            shp[r0:r0 + 128] = (
                sh[r0:r0 + 128] * _E3_SCALE
            ).astype(ml_dtypes.float8_e3m4).view(np.int8)
            vs[r0:r0 + 128] = v[c * _FSH + r0:c * _FSH + r0 + 128] / _E3_SCALE
        # [oct, chunk-in-oct, partition, batch] -> [oct, partition, ...]
        xqc = np.ascontiguousarray(
            shp.reshape(_NOCT, 8, 128, _B).transpose(0, 2, 1, 3)
        ).reshape(_NOCT, 128, 8 * _B)
        vmat = np.ascontiguousarray(vs.reshape(_NCHP, 128).T)   # [128, NCHP]
        in_maps.append({
            "xq": xqc,
            "vpe": vmat.astype(ml_dtypes.bfloat16),
            "vdve": vmat.astype(np.float32),
        })
    r = run_bass_kernel_spmd(nc, in_maps, core_ids=list(range(_NCORES)))
    kernel._last = r
    acc = np.zeros(_B, np.float64)
    for i in range(_NCORES):
        acc += r.results[i]["y"].reshape(-1).astype(np.float64)
    return (acc + const).astype(np.float32)
